# revision 1
# baseline (speedup 1.0000x reference)
"""LookupConv2d Trainium2 kernel.

Math: out = conv2d(x, W), W[o] = sum_s coeff[o,s] * dictionary[idx[o,s]].
Factorization: W = M @ D where M[o,d] = sum_{s: idx[o,s]=d} coeff[o,s] is a
(512, 100) scatter of the coefficients.  Then
    out = M @ conv2d(x, dictionary)
i.e. a 100-channel conv (23 GFLOP) followed by a 1x1 512x100 mix (5 GFLOP)
instead of a 512-channel conv (118 GFLOP) -- 4.2x fewer FLOPs.

Precision: the TensorE f32r mode streams 1 row/cycle (4x faster than fp32
mode) but rounds operands to 12 significant bits (RNE, measured on HW).
We split x and the dictionary into exact 12-bit halves (xh = top 12 bits,
xl = remainder, both f32r-invariant) and accumulate
    xh*wh + xl*wh + xh*wl
in fp32 PSUM -- full fp32-class accuracy (only xl*wl ~ 2^-24 dropped) at
3 cycles/row instead of fp32 mode's 4.  The small 1x1 mix stays in native
fp32 mode.

Sharding: data-parallel over batch N=16 -> 2 images per core on 8 cores.
dictionary (as [128,100] lhsT tap matrices) and M^T are replicated.
"""

import numpy as np

N_CORES = 8
IMGS_PER_CORE = 2
CIN = 256
COUT = 512
NDICT = 100
H = W = 56
HP = WP = 58  # padded
ROWS_PER_TILE = 8
N_TILES = H // ROWS_PER_TILE  # 7
FREE = ROWS_PER_TILE * W  # 448
S = 3  # lookup sparsity

TRACE = False  # set by test.py to get a profile
_LAST_RESULTS = {}  # test.py reads exec_time_ns from here


def split12(a):
    """Exact split a = hi + lo with <=12 significant bits each (a ~ N(0,1),
    so no denormal/overflow concerns).  Both halves pass through the f32r
    12-bit RNE rounding unchanged."""
    a = np.ascontiguousarray(a, dtype=np.float32)
    hi = (a.view(np.uint32) & np.uint32(0xFFFFF000)).view(np.float32)
    lo = (a - hi).astype(np.float32)
    return hi, lo


def _build_program():
    import concourse.bacc as bacc
    import concourse.mybir as mybir
    import concourse.tile as tile

    f32 = mybir.dt.float32
    f32r = mybir.dt.float32r

    nc = bacc.Bacc("TRN2", target_bir_lowering=False, debug=False)

    xh_d = nc.dram_tensor("xh", (IMGS_PER_CORE, CIN, HP, WP), f32,
                          kind="ExternalInput")
    xl_d = nc.dram_tensor("xl", (IMGS_PER_CORE, CIN, HP, WP), f32,
                          kind="ExternalInput")
    wh_d = nc.dram_tensor("wh", (128, 2 * 9 * NDICT), f32, kind="ExternalInput")
    wl_d = nc.dram_tensor("wl", (128, 2 * 9 * NDICT), f32, kind="ExternalInput")
    mh_d = nc.dram_tensor("mh", (NDICT, COUT), f32, kind="ExternalInput")
    ml_d = nc.dram_tensor("ml", (NDICT, COUT), f32, kind="ExternalInput")
    out_d = nc.dram_tensor("out", (IMGS_PER_CORE, COUT, H, W), f32,
                           kind="ExternalOutput")

    # row chunks of the padded input: first 10 rows, then 6x8 -- tile t only
    # needs chunks 0..t so compute starts after the first chunk lands
    row_chunks = [(0, 10)] + [(10 + 8 * k, 8) for k in range(6)]

    with tile.TileContext(nc) as tc:
        with (
            tc.tile_pool(name="consts", bufs=1) as consts,
            tc.tile_pool(name="xpool", bufs=1) as xpool,
            tc.tile_pool(name="ypool", bufs=3) as ypool,
            tc.tile_pool(name="opool", bufs=8) as opool,
            tc.tile_pool(name="psum_y", bufs=2, space="PSUM") as psum_y_pool,
            tc.tile_pool(name="psum_o", bufs=4, space="PSUM") as psum_o_pool,
        ):
            wh_sb = consts.tile([128, 2 * 9 * NDICT], f32r)
            nc.sync.dma_start(wh_sb[:], wh_d[:].bitcast(f32r))
            wl_sb = consts.tile([128, 2 * 9 * NDICT], f32r)
            nc.sync.dma_start(wl_sb[:], wl_d[:].bitcast(f32r))
            mh_sb = consts.tile([NDICT, COUT], f32r)
            nc.sync.dma_start(mh_sb[:], mh_d[:].bitcast(f32r))
            ml_sb = consts.tile([NDICT, COUT], f32r)
            nc.sync.dma_start(ml_sb[:], ml_d[:].bitcast(f32r))

            # [128 cin-in-block, img, cblk, hp, wp]
            xh_sb = xpool.tile([128, IMGS_PER_CORE, 2, HP, WP], f32r,
                               tag="xh_sb")
            xl_sb = xpool.tile([128, IMGS_PER_CORE, 2, HP, WP], f32r,
                               tag="xl_sb")
            xh_v = xh_d.rearrange("i (b c) h w -> c i b h w", c=128)
            xl_v = xl_d.rearrange("i (b c) h w -> c i b h w", c=128)
            for img in range(IMGS_PER_CORE):
                for r0, nr in row_chunks:
                    for cb in range(2):
                        nc.sync.dma_start(
                            xh_sb[:, img, cb, r0:r0 + nr, :],
                            xh_v[:, img, cb, r0:r0 + nr, :].bitcast(f32r))
                        nc.sync.dma_start(
                            xl_sb[:, img, cb, r0:r0 + nr, :],
                            xl_v[:, img, cb, r0:r0 + nr, :].bitcast(f32r))

            out_v = out_d.rearrange("i (b o) h w -> i b o (h w)", o=128)

            n_mm = 3 * 18

            def emit_conv(img, h0):
                py = psum_y_pool.tile([NDICT, FREE], f32)
                k = 0
                for cb in range(2):
                    for ti in range(3):
                        for tj in range(3):
                            tap = slice((cb * 9 + ti * 3 + tj) * NDICT,
                                        (cb * 9 + ti * 3 + tj + 1) * NDICT)
                            rh = (slice(None), img, cb,
                                  slice(h0 + ti, h0 + ti + ROWS_PER_TILE),
                                  slice(tj, tj + W))
                            for lhsT, rhs in (
                                (wh_sb[:, tap], xh_sb[rh]),
                                (wh_sb[:, tap], xl_sb[rh]),
                                (wl_sb[:, tap], xh_sb[rh]),
                            ):
                                nc.tensor.matmul(
                                    py[:], lhsT, rhs,
                                    start=(k == 0), stop=(k == n_mm - 1))
                                k += 1
                return py

            def emit_mix(py, img, h0):
                # Veltkamp split y = yh + yl into 12-bit halves (pure fp32
                # arithmetic; values are exactly f32r-representable so the
                # matmul's internal rounding is the identity)
                t_sb = ypool.tile([NDICT, FREE], f32, tag="t")
                big = ypool.tile([NDICT, FREE], f32, tag="big")
                yh = ypool.tile([NDICT, FREE], f32r, tag="yh")
                yl = ypool.tile([NDICT, FREE], f32r, tag="yl")
                nc.scalar.mul(t_sb[:], py[:], 4097.0)
                nc.vector.tensor_sub(big[:], t_sb[:], py[:])
                nc.vector.tensor_sub(yh[:], t_sb[:], big[:])
                nc.vector.tensor_sub(yl[:], py[:], yh[:])
                for ob in range(4):
                    obs = slice(ob * 128, (ob + 1) * 128)
                    po = psum_o_pool.tile([128, FREE], f32)
                    nc.tensor.matmul(po[:], mh_sb[:, obs], yh[:],
                                     start=True, stop=False)
                    nc.tensor.matmul(po[:], ml_sb[:, obs], yh[:],
                                     start=False, stop=False)
                    nc.tensor.matmul(po[:], mh_sb[:, obs], yl[:],
                                     start=False, stop=True)
                    o_sb = opool.tile([128, FREE], f32)
                    if ob % 2 == 0:
                        nc.vector.tensor_copy(o_sb[:], po[:])
                    else:
                        nc.scalar.copy(o_sb[:], po[:])
                    nc.sync.dma_start(
                        out_v[img, ob, :, h0 * W:h0 * W + FREE], o_sb[:])

            # software-pipeline by one tile: PE runs tile i's conv while
            # ACT/DVE run tile i-1's Veltkamp split, so the mix matmuls are
            # ready when PE gets to them
            pending = None
            for img in range(IMGS_PER_CORE):
                for t in range(N_TILES):
                    h0 = t * ROWS_PER_TILE
                    py = emit_conv(img, h0)
                    if pending is not None:
                        emit_mix(*pending)
                    pending = (py, img, h0)
            emit_mix(*pending)

    nc.compile()
    return nc


_NC_CACHE = None


def kernel(x, dictionary, lookup_indices, lookup_coefficients):
    global _NC_CACHE
    from concourse import bass_utils

    x = np.asarray(x, dtype=np.float32)
    dictionary = np.asarray(dictionary, dtype=np.float32)
    idx = np.asarray(lookup_indices).astype(np.int64)
    coef = np.asarray(lookup_coefficients, dtype=np.float32)

    # M^T[d, o] = sum_s coeff[o, s] * [idx[o, s] == d]
    mt = np.zeros((NDICT, COUT), np.float32)
    np.add.at(mt, (idx.reshape(-1),
                   np.repeat(np.arange(COUT), S)), coef.reshape(-1))

    # wt[c_in_block, (cblk, ti, tj, d)] = dictionary[d, cblk*128+c, ti, tj]
    wt = np.ascontiguousarray(
        dictionary.reshape(NDICT, 2, 128, 3, 3).transpose(2, 1, 3, 4, 0)
    ).reshape(128, 2 * 9 * NDICT)
    wh, wl = split12(wt)
    mh, ml = split12(mt)

    xp = np.pad(x, ((0, 0), (0, 0), (1, 1), (1, 1)))
    xp = np.ascontiguousarray(
        xp.reshape(N_CORES, IMGS_PER_CORE, CIN, HP, WP))
    xh, xl = split12(xp)

    if _NC_CACHE is None:
        _NC_CACHE = _build_program()
    nc = _NC_CACHE

    in_maps = [{"xh": xh[i], "xl": xl[i], "wh": wh, "wl": wl,
                "mh": mh, "ml": ml} for i in range(N_CORES)]
    try:
        res = bass_utils.run_bass_kernel_spmd(
            nc, in_maps, core_ids=list(range(N_CORES)), trace=TRACE)
    except ModuleNotFoundError:
        # no axon NTFF profile hook in this environment
        res = bass_utils.run_bass_kernel_spmd(
            nc, in_maps, core_ids=list(range(N_CORES)), trace=False)
    _LAST_RESULTS["res"] = res

    out = np.concatenate([r["out"] for r in res.results], axis=0)
    return out.reshape(16, COUT, H, W)



# revision 2
# speedup vs baseline: 2.6479x; 2.6479x over previous
"""LookupConv2d Trainium2 kernel.

Math: out = conv2d(x, W), W[o] = sum_s coeff[o,s] * dictionary[idx[o,s]].
Factorization: W = M @ D where M[o,d] = sum_{s: idx[o,s]=d} coeff[o,s] is a
(512, 100) scatter of the coefficients.  Then
    out = M @ conv2d(x, dictionary)
i.e. a 100-channel conv (23 GFLOP) followed by a 1x1 512x100 mix (5 GFLOP)
instead of a 512-channel conv (118 GFLOP) -- 4.2x fewer FLOPs.

Precision: single-pass bf16.  All matmul operands are bf16 (streamed at
1 column/cycle on TensorE, same rate as f32r), accumulation is fp32 in
PSUM.  Simulated end-to-end rel err 3.6e-3, well under the 2e-2 gate.
I/O is bf16 as well (input 3.45 MB, output 6.4 MB per core), halving HBM
traffic vs fp32.

Sharding: data-parallel over batch N=16 -> 2 images per core on 8 cores.
dictionary (as [128, 1800] bf16 tap matrices) and M^T are replicated.
"""

import numpy as np
import ml_dtypes

N_CORES = 8
IMGS_PER_CORE = 2
CIN = 256
COUT = 512
NDICT = 100
H = W = 56
HP = WP = 58  # padded
ROWS_PER_TILE = 8
N_TILES = H // ROWS_PER_TILE  # 7
FREE = ROWS_PER_TILE * W  # 448
S = 3  # lookup sparsity

TRACE = False  # set by test.py to get a profile
_LAST_RESULTS = {}  # test.py reads exec_time_ns from here


def _build_program():
    import concourse.bacc as bacc
    import concourse.mybir as mybir
    import concourse.tile as tile

    f32 = mybir.dt.float32
    bf16 = mybir.dt.bfloat16

    nc = bacc.Bacc("TRN2", target_bir_lowering=False, debug=False)

    x_d = nc.dram_tensor("x", (IMGS_PER_CORE, CIN, HP, WP), bf16,
                         kind="ExternalInput")
    w_d = nc.dram_tensor("w", (128, 2 * 9 * NDICT), bf16, kind="ExternalInput")
    m_d = nc.dram_tensor("m", (NDICT, COUT), bf16, kind="ExternalInput")
    out_d = nc.dram_tensor("out", (IMGS_PER_CORE, COUT, H, W), bf16,
                           kind="ExternalOutput")

    # row chunks of the padded input: first 10 rows, then 6x8 -- tile t only
    # needs chunks 0..t so compute starts after the first chunk lands
    row_chunks = [(0, 10)] + [(10 + 8 * k, 8) for k in range(6)]

    with tile.TileContext(nc) as tc:
        with (
            tc.tile_pool(name="consts", bufs=1) as consts,
            tc.tile_pool(name="xpool", bufs=1) as xpool,
            tc.tile_pool(name="ypool", bufs=3) as ypool,
            tc.tile_pool(name="opool", bufs=8) as opool,
            tc.tile_pool(name="psum_y", bufs=2, space="PSUM") as psum_y_pool,
            tc.tile_pool(name="psum_o", bufs=4, space="PSUM") as psum_o_pool,
        ):
            w_sb = consts.tile([128, 2 * 9 * NDICT], bf16)
            nc.sync.dma_start(w_sb[:], w_d[:])
            m_sb = consts.tile([NDICT, COUT], bf16)
            nc.sync.dma_start(m_sb[:], m_d[:])

            # [128 cin-in-block, img, cblk, hp, wp]
            x_sb = xpool.tile([128, IMGS_PER_CORE, 2, HP, WP], bf16,
                              tag="x_sb")
            x_v = x_d.rearrange("i (b c) h w -> c i b h w", c=128)
            for img in range(IMGS_PER_CORE):
                for r0, nr in row_chunks:
                    for cb in range(2):
                        nc.sync.dma_start(
                            x_sb[:, img, cb, r0:r0 + nr, :],
                            x_v[:, img, cb, r0:r0 + nr, :])

            out_v = out_d.rearrange("i (b o) h w -> i b o (h w)", o=128)

            n_mm = 18

            def emit_conv(img, h0):
                py = psum_y_pool.tile([NDICT, FREE], f32)
                k = 0
                for cb in range(2):
                    for ti in range(3):
                        for tj in range(3):
                            tap = slice((cb * 9 + ti * 3 + tj) * NDICT,
                                        (cb * 9 + ti * 3 + tj + 1) * NDICT)
                            rh = (slice(None), img, cb,
                                  slice(h0 + ti, h0 + ti + ROWS_PER_TILE),
                                  slice(tj, tj + W))
                            nc.tensor.matmul(
                                py[:], w_sb[:, tap], x_sb[rh],
                                start=(k == 0), stop=(k == n_mm - 1))
                            k += 1
                return py

            def emit_mix(py, img, h0):
                y_sb = ypool.tile([NDICT, FREE], bf16, tag="y")
                nc.vector.tensor_copy(y_sb[:], py[:])
                for ob in range(4):
                    obs = slice(ob * 128, (ob + 1) * 128)
                    po = psum_o_pool.tile([128, FREE], f32)
                    nc.tensor.matmul(po[:], m_sb[:, obs], y_sb[:],
                                     start=True, stop=True)
                    o_sb = opool.tile([128, FREE], bf16)
                    if ob % 2 == 0:
                        nc.vector.tensor_copy(o_sb[:], po[:])
                    else:
                        nc.scalar.copy(o_sb[:], po[:])
                    nc.sync.dma_start(
                        out_v[img, ob, :, h0 * W:h0 * W + FREE], o_sb[:])

            # software-pipeline by one tile: PE runs tile i's conv while
            # ACT/DVE copy tile i-1's PSUM out, so the mix matmuls are
            # ready when PE gets to them
            pending = None
            for img in range(IMGS_PER_CORE):
                for t in range(N_TILES):
                    h0 = t * ROWS_PER_TILE
                    py = emit_conv(img, h0)
                    if pending is not None:
                        emit_mix(*pending)
                    pending = (py, img, h0)
            emit_mix(*pending)

    nc.compile()
    return nc


_NC_CACHE = None


def kernel(x, dictionary, lookup_indices, lookup_coefficients):
    global _NC_CACHE
    from concourse import bass_utils

    x = np.asarray(x, dtype=np.float32)
    dictionary = np.asarray(dictionary, dtype=np.float32)
    idx = np.asarray(lookup_indices).astype(np.int64)
    coef = np.asarray(lookup_coefficients, dtype=np.float32)

    # M^T[d, o] = sum_s coeff[o, s] * [idx[o, s] == d]
    mt = np.zeros((NDICT, COUT), np.float32)
    np.add.at(mt, (idx.reshape(-1),
                   np.repeat(np.arange(COUT), S)), coef.reshape(-1))

    # wt[c_in_block, (cblk, ti, tj, d)] = dictionary[d, cblk*128+c, ti, tj]
    wt = np.ascontiguousarray(
        dictionary.reshape(NDICT, 2, 128, 3, 3).transpose(2, 1, 3, 4, 0)
    ).reshape(128, 2 * 9 * NDICT)

    xp = np.pad(x, ((0, 0), (0, 0), (1, 1), (1, 1)))
    xp = np.ascontiguousarray(
        xp.reshape(N_CORES, IMGS_PER_CORE, CIN, HP, WP))

    bf = ml_dtypes.bfloat16
    xb = xp.astype(bf)
    wb = wt.astype(bf)
    mb = mt.astype(bf)

    if _NC_CACHE is None:
        _NC_CACHE = _build_program()
    nc = _NC_CACHE

    in_maps = [{"x": xb[i], "w": wb, "m": mb} for i in range(N_CORES)]
    try:
        res = bass_utils.run_bass_kernel_spmd(
            nc, in_maps, core_ids=list(range(N_CORES)), trace=TRACE)
    except ModuleNotFoundError:
        # no axon NTFF profile hook in this environment
        res = bass_utils.run_bass_kernel_spmd(
            nc, in_maps, core_ids=list(range(N_CORES)), trace=False)
    _LAST_RESULTS["res"] = res

    out = np.concatenate([np.asarray(r["out"]) for r in res.results], axis=0)
    return out.reshape(16, COUT, H, W).astype(np.float32)


# revision 20
# speedup vs baseline: 2.8347x; 1.0705x over previous
"""LookupConv2d Trainium2 kernel.

Math: out = conv2d(x, W), W[o] = sum_s coeff[o,s] * dictionary[idx[o,s]].
Factorization: W = M @ D where M[o,d] = sum_{s: idx[o,s]=d} coeff[o,s] is a
(512, 100) scatter of the coefficients.  Then
    out = M @ conv2d(x, dictionary)
i.e. a 100-channel conv (23 GFLOP) followed by a 1x1 512x100 mix (5 GFLOP)
instead of a 512-channel conv (118 GFLOP) -- 4.2x fewer FLOPs.

Precision: single-pass bf16.  All matmul operands are bf16 (streamed at
1 column/cycle on TensorE, same rate as f32r), accumulation is fp32 in
PSUM.  Simulated end-to-end rel err 3.6e-3, well under the 2e-2 gate.
I/O is bf16 as well (input 3.45 MB, output 6.4 MB per core), halving HBM
traffic vs fp32.

Sharding: data-parallel over batch N=16 -> 2 images per core on 8 cores.
dictionary (as [128, 1800] bf16 tap matrices) and M^T are replicated.
"""

import numpy as np
import ml_dtypes

N_CORES = 8
IMGS_PER_CORE = 2
CIN = 256
COUT = 512
NDICT = 100
H = W = 56
HP = WP = 58  # padded
ROWS_PER_TILE = 8
N_TILES = H // ROWS_PER_TILE  # 7
FREE = ROWS_PER_TILE * W  # 448
S = 3  # lookup sparsity

TRACE = False  # set by test.py to get a profile
_LAST_RESULTS = {}  # test.py reads exec_time_ns from here


def _build_program(head="A", tail="pairs-pairs-pairs", tiles1="8"):
    import concourse.bacc as bacc
    import concourse.mybir as mybir
    import concourse.tile as tile

    f32 = mybir.dt.float32
    bf16 = mybir.dt.bfloat16

    nc = bacc.Bacc("TRN2", target_bir_lowering=False, debug=False)

    x_d = nc.dram_tensor("x", (IMGS_PER_CORE, CIN, HP, WP), bf16,
                         kind="ExternalInput")
    w_d = nc.dram_tensor("w", (128, 2 * 9 * NDICT), bf16, kind="ExternalInput")
    m_d = nc.dram_tensor("m", (NDICT, COUT), bf16, kind="ExternalInput")
    out_d = nc.dram_tensor("out", (IMGS_PER_CORE, COUT, H, W), bf16,
                           kind="ExternalOutput")

    with tile.TileContext(nc) as tc:
        with (
            tc.tile_pool(name="consts", bufs=1) as consts,
            tc.tile_pool(name="xpool", bufs=1) as xpool,
            tc.tile_pool(name="ypool", bufs=3) as ypool,
            tc.tile_pool(name="opool", bufs=3) as opool,
            tc.tile_pool(name="psum_y", bufs=2, space="PSUM") as psum_y_pool,
            tc.tile_pool(name="psum_o", bufs=6, space="PSUM") as psum_o_pool,
        ):
            w_sb = consts.tile([128, 2 * 9 * NDICT], bf16)
            m_sb = consts.tile([NDICT, COUT], bf16)
            # [128 cin-in-block, img, cblk, hp, wp]
            x_sb = xpool.tile([128, IMGS_PER_CORE, 2, HP, WP], bf16,
                              tag="x_sb")
            x_v = x_d.rearrange("i (b c) h w -> c i b h w", c=128)

            # prologue DMAs, ordered by when compute needs them: the first
            # 3 conv matmuls only need the first 3 cb=0 taps and rows 0..9
            # of image 0.  Few large transfers keep the HWDGE (fixed
            # ~625ns per dma_start) far from saturation.
            def d_x(img, cb, r0, r1):
                if cb is None:
                    nc.sync.dma_start(x_sb[:, img, :, r0:r1, :],
                                      x_v[:, img, :, r0:r1, :])
                else:
                    nc.sync.dma_start(x_sb[:, img, cb, r0:r1, :],
                                      x_v[:, img, cb, r0:r1, :])

            def d_w(c0, c1):
                nc.sync.dma_start(w_sb[:, c0 * NDICT:c1 * NDICT],
                                  w_d[:, c0 * NDICT:c1 * NDICT])

            def d_m():
                nc.sync.dma_start(m_sb[:], m_d[:])

            heads = {
                # x img0 rows first-chunk / weight pieces / m, in need order
                "A": lambda: (d_x(0, 0, 0, 6), d_w(0, 3), d_x(0, 1, 0, 6),
                              d_x(0, None, 6, 10), d_w(3, 9), d_w(9, 18),
                              d_m()),
                "B": lambda: (d_x(0, 0, 0, 6), d_w(0, 3), d_x(0, 1, 0, 6),
                              d_w(3, 9), d_w(9, 18), d_x(0, None, 6, 10),
                              d_m()),
                "C": lambda: (d_x(0, 0, 0, 6), d_w(0, 3), d_w(3, 9),
                              d_x(0, 1, 0, 6), d_w(9, 18),
                              d_x(0, None, 6, 10), d_m()),
                "D": lambda: (d_x(0, 0, 0, 10), d_w(0, 3), d_x(0, 1, 0, 10),
                              d_w(3, 9), d_w(9, 18), d_m()),
            }
            heads[head]()
            late_chunks = [(10, 24), (34, 24)]
            for r0, nr in late_chunks:
                d_x(0, None, r0, r0 + nr)
            for r0, nr in [(0, 10)] + late_chunks:
                d_x(1, None, r0, r0 + nr)

            # [img, o-in-block, oblk, (h w)] -- partition dim is o
            out_v = out_d.rearrange("i (b o) h w -> i o b (h w)", o=128)

            def emit_conv(img, h0, nr):
                free = nr * W
                py = psum_y_pool.tile([NDICT, free], f32, tag="py")
                k = 0
                for cb in range(2):
                    for ti in range(3):
                        for tj in range(3):
                            tap = slice((cb * 9 + ti * 3 + tj) * NDICT,
                                        (cb * 9 + ti * 3 + tj + 1) * NDICT)
                            rh = (slice(None), img, cb,
                                  slice(h0 + ti, h0 + ti + nr),
                                  slice(tj, tj + W))
                            nc.tensor.matmul(
                                py[:], w_sb[:, tap], x_sb[rh],
                                start=(k == 0), stop=(k == 17))
                            k += 1
                return py

            def emit_mix(py, img, h0, nr, mode="pairs"):
                free = nr * W
                y_sb = ypool.tile([NDICT, free], bf16, tag="y")
                nc.vector.tensor_copy(y_sb[:], py[:])
                o_sb = opool.tile([128, 4, free], bf16, tag="o")
                for ob in range(4):
                    obs = slice(ob * 128, (ob + 1) * 128)
                    po = psum_o_pool.tile([128, free], f32, tag="po")
                    nc.tensor.matmul(po[:], m_sb[:, obs], y_sb[:],
                                     start=True, stop=True)
                    if ob % 2 == 0:
                        nc.vector.tensor_copy(o_sb[:, ob, :], po[:])
                    else:
                        nc.scalar.copy(o_sb[:, ob, :], po[:])
                    if mode == "split":
                        nc.sync.dma_start(
                            out_v[img, :, ob, h0 * W:h0 * W + free],
                            o_sb[:, ob, :])
                    elif mode == "pairs" and ob % 2 == 1:
                        # pairwise DMAs decouple the transfer start from the
                        # slowest of all 4 copies without doubling HWDGE load
                        nc.sync.dma_start(
                            out_v[img, :, ob - 1:ob + 1, h0 * W:h0 * W + free],
                            o_sb[:, ob - 1:ob + 1, :])
                if mode == "merged":
                    nc.sync.dma_start(
                        out_v[img, :, :, h0 * W:h0 * W + free], o_sb[:])

            # 8-row tiles, except: the first tile is 2x4 rows (compute can
            # start after only 6 input rows + 3 taps land, and the cold
            # PE-clock ramp is spent on cheap matmuls); optionally the
            # global last tile is 2x4 rows so the kernel tail (copies + DMA
            # of the last tile, which cannot overlap compute) is shallower
            tiles = {0: [(0, 4), (4, 4)] + [(8 + 8 * t, 8) for t in range(6)]}
            if tiles1 == "44":
                tiles[1] = ([(8 * t, 8) for t in range(6)]
                            + [(48, 4), (52, 4)])
            else:
                tiles[1] = [(8 * t, 8) for t in range(7)]
            n_total = len(tiles[0]) + len(tiles[1])

            # tail= "<mid>-<lastk>-<last>": DMA mode for mid tiles, for the
            # 2 next-to-last tiles, and for the final tile
            mid_mode, lastk_mode, last_mode = tail.split("-")

            # software-pipeline by one tile: PE runs tile i's conv while
            # ACT/DVE copy tile i-1's PSUM out, so the mix matmuls are
            # ready when PE gets to them
            pending = None
            emitted = 0
            for img in range(IMGS_PER_CORE):
                for h0, nr in tiles[img]:
                    py = emit_conv(img, h0, nr)
                    if pending is not None:
                        emitted += 1
                        mode = (mid_mode if emitted < n_total - 2
                                else lastk_mode)
                        emit_mix(*pending, mode=mode)
                    pending = (py, img, h0, nr)
            emit_mix(*pending, mode=last_mode)

    nc.compile()
    return nc


_NC_CACHE = None


def kernel(x, dictionary, lookup_indices, lookup_coefficients):
    global _NC_CACHE
    from concourse import bass_utils

    x = np.asarray(x, dtype=np.float32)
    dictionary = np.asarray(dictionary, dtype=np.float32)
    idx = np.asarray(lookup_indices).astype(np.int64)
    coef = np.asarray(lookup_coefficients, dtype=np.float32)

    # M^T[d, o] = sum_s coeff[o, s] * [idx[o, s] == d]
    mt = np.zeros((NDICT, COUT), np.float32)
    np.add.at(mt, (idx.reshape(-1),
                   np.repeat(np.arange(COUT), S)), coef.reshape(-1))

    # wt[c_in_block, (cblk, ti, tj, d)] = dictionary[d, cblk*128+c, ti, tj]
    wt = np.ascontiguousarray(
        dictionary.reshape(NDICT, 2, 128, 3, 3).transpose(2, 1, 3, 4, 0)
    ).reshape(128, 2 * 9 * NDICT)

    xp = np.pad(x, ((0, 0), (0, 0), (1, 1), (1, 1)))
    xp = np.ascontiguousarray(
        xp.reshape(N_CORES, IMGS_PER_CORE, CIN, HP, WP))

    bf = ml_dtypes.bfloat16
    xb = xp.astype(bf)
    wb = wt.astype(bf)
    mb = mt.astype(bf)

    if _NC_CACHE is None:
        _NC_CACHE = _build_program()
    nc = _NC_CACHE

    in_maps = [{"x": xb[i], "w": wb, "m": mb} for i in range(N_CORES)]
    try:
        res = bass_utils.run_bass_kernel_spmd(
            nc, in_maps, core_ids=list(range(N_CORES)), trace=TRACE)
    except ModuleNotFoundError:
        # no axon NTFF profile hook in this environment
        res = bass_utils.run_bass_kernel_spmd(
            nc, in_maps, core_ids=list(range(N_CORES)), trace=False)
    _LAST_RESULTS["res"] = res

    out = np.concatenate([np.asarray(r["out"]) for r in res.results], axis=0)
    return out.reshape(16, COUT, H, W).astype(np.float32)


# revision 38
# speedup vs baseline: 2.8735x; 1.0137x over previous
"""LookupConv2d Trainium2 kernel.

Math: out = conv2d(x, W), W[o] = sum_s coeff[o,s] * dictionary[idx[o,s]].
Factorization: W = M @ D where M[o,d] = sum_{s: idx[o,s]=d} coeff[o,s] is a
(512, 100) scatter of the coefficients.  Then
    out = M @ conv2d(x, dictionary)
i.e. a 100-channel conv (23 GFLOP) followed by a 1x1 512x100 mix (5 GFLOP)
instead of a 512-channel conv (118 GFLOP) -- 4.2x fewer FLOPs.

Precision: single-pass bf16.  All matmul operands are bf16 (streamed at
1 column/cycle on TensorE, same rate as f32r), accumulation is fp32 in
PSUM.  Simulated end-to-end rel err 3.6e-3, well under the 2e-2 gate.
I/O is bf16 as well (input 3.45 MB, output 6.4 MB per core), halving HBM
traffic vs fp32.

Sharding: data-parallel over batch N=16 -> 2 images per core on 8 cores.
dictionary (as [128, 1800] bf16 tap matrices) and M^T are replicated.
"""

import numpy as np
import ml_dtypes

N_CORES = 8
IMGS_PER_CORE = 2
CIN = 256
COUT = 512
NDICT = 100
H = W = 56
HP = WP = 58  # padded
ROWS_PER_TILE = 8
N_TILES = H // ROWS_PER_TILE  # 7
FREE = ROWS_PER_TILE * W  # 448
S = 3  # lookup sparsity

TRACE = False  # set by test.py to get a profile
_LAST_RESULTS = {}  # test.py reads exec_time_ns from here


def _tiles(tiles1="8"):
    """Row-tile grid per image.  img0 starts with two 4-row tiles so
    compute can begin after only 6 input rows + 3 taps land; img1's tail
    can end with small tiles to shorten the kernel tail."""
    t0 = [(0, 4), (4, 4)] + [(8 + 8 * t, 8) for t in range(6)]
    if tiles1 == "44":
        t1 = [(8 * t, 8) for t in range(6)] + [(48, 4), (52, 4)]
    elif tiles1 == "62":
        t1 = [(8 * t, 8) for t in range(6)] + [(48, 6), (54, 2)]
    else:
        t1 = [(8 * t, 8) for t in range(7)]
    return {0: t0, 1: t1}


def _build_program(head="C", tail="pairs-merged-merged", tiles1="44",
                   fast_mode="none"):
    import concourse.bacc as bacc
    import concourse.mybir as mybir
    import concourse.tile as tile

    f32 = mybir.dt.float32
    bf16 = mybir.dt.bfloat16

    nc = bacc.Bacc("TRN2", target_bir_lowering=False, debug=False)

    x_d = nc.dram_tensor("x", (IMGS_PER_CORE, CIN, HP, WP), bf16,
                         kind="ExternalInput")
    w_d = nc.dram_tensor("w", (128, 2 * 9 * NDICT), bf16, kind="ExternalInput")
    m_d = nc.dram_tensor("m", (NDICT, COUT), bf16, kind="ExternalInput")
    # tile-major output layout out[o, 4*(img*3136 + h0*56) + ob*free + px]:
    # every out-DMA writes a per-partition-contiguous run (>=512 B even for
    # tiny tail tiles), so all transfers go at full HBM line rate.  The
    # host untangles this for free.
    out_d = nc.dram_tensor("out", (128, 4 * H * W * IMGS_PER_CORE), bf16,
                           kind="ExternalOutput")

    with tile.TileContext(nc) as tc:
        with (
            tc.tile_pool(name="consts", bufs=1) as consts,
            tc.tile_pool(name="xpool", bufs=1) as xpool,
            tc.tile_pool(name="ypool", bufs=3) as ypool,
            tc.tile_pool(name="opool", bufs=3) as opool,
            tc.tile_pool(name="psum_y", bufs=2, space="PSUM") as psum_y_pool,
            tc.tile_pool(name="psum_o", bufs=6, space="PSUM") as psum_o_pool,
        ):
            w_sb = consts.tile([128, 2 * 9 * NDICT], bf16)
            m_sb = consts.tile([NDICT, COUT], bf16)
            # [128 cin-in-block, img, cblk, hp, wp]
            x_sb = xpool.tile([128, IMGS_PER_CORE, 2, HP, WP], bf16,
                              tag="x_sb")
            x_v = x_d.rearrange("i (b c) h w -> c i b h w", c=128)

            # prologue DMAs, ordered by when compute needs them: the first
            # 3 conv matmuls only need the first 3 cb=0 taps and rows 0..9
            # of image 0.  Few large transfers keep the HWDGE (fixed
            # ~625ns per dma_start) far from saturation.
            def d_x(img, cb, r0, r1):
                if cb is None:
                    nc.sync.dma_start(x_sb[:, img, :, r0:r1, :],
                                      x_v[:, img, :, r0:r1, :])
                else:
                    nc.sync.dma_start(x_sb[:, img, cb, r0:r1, :],
                                      x_v[:, img, cb, r0:r1, :])

            def d_w(c0, c1):
                nc.sync.dma_start(w_sb[:, c0 * NDICT:c1 * NDICT],
                                  w_d[:, c0 * NDICT:c1 * NDICT])

            def d_m():
                nc.sync.dma_start(m_sb[:], m_d[:])

            heads = {
                # x img0 rows first-chunk / weight pieces / m, in need order
                "A": lambda: (d_x(0, 0, 0, 6), d_w(0, 3), d_x(0, 1, 0, 6),
                              d_x(0, None, 6, 10), d_w(3, 9), d_w(9, 18),
                              d_m()),
                "B": lambda: (d_x(0, 0, 0, 6), d_w(0, 3), d_x(0, 1, 0, 6),
                              d_w(3, 9), d_w(9, 18), d_x(0, None, 6, 10),
                              d_m()),
                "C": lambda: (d_x(0, 0, 0, 6), d_w(0, 3), d_w(3, 9),
                              d_x(0, 1, 0, 6), d_w(9, 18),
                              d_x(0, None, 6, 10), d_m()),
                "D": lambda: (d_x(0, 0, 0, 10), d_w(0, 3), d_x(0, 1, 0, 10),
                              d_w(3, 9), d_w(9, 18), d_m()),
                "E": lambda: (d_x(0, 0, 0, 6), d_w(0, 3), d_x(0, 1, 0, 6),
                              d_w(3, 9), d_x(0, None, 6, 10), d_w(9, 18),
                              d_m()),
                "F": lambda: (d_x(0, 0, 0, 6), d_w(0, 6), d_x(0, 1, 0, 6),
                              d_w(6, 12), d_x(0, None, 6, 10), d_w(12, 18),
                              d_m()),
            }
            heads[head]()
            late_chunks = [(10, 24), (34, 24)]
            for r0, nr in late_chunks:
                d_x(0, None, r0, r0 + nr)
            for r0, nr in [(0, 10)] + late_chunks:
                d_x(1, None, r0, r0 + nr)



            def emit_conv(img, h0, nr):
                free = nr * W
                py = psum_y_pool.tile([NDICT, free], f32, tag="py")
                k = 0
                for cb in range(2):
                    for ti in range(3):
                        for tj in range(3):
                            tap = slice((cb * 9 + ti * 3 + tj) * NDICT,
                                        (cb * 9 + ti * 3 + tj + 1) * NDICT)
                            rh = (slice(None), img, cb,
                                  slice(h0 + ti, h0 + ti + nr),
                                  slice(tj, tj + W))
                            nc.tensor.matmul(
                                py[:], w_sb[:, tap], x_sb[rh],
                                start=(k == 0), stop=(k == 17))
                            k += 1
                return py

            def emit_mix(py, img, h0, nr, mode="pairs", fast=False):
                free = nr * W
                half = free // 2
                off = 4 * (img * H * W + h0 * W)
                y_sb = ypool.tile([NDICT, free], bf16, tag="y")
                if fast and fast_mode == "yout":
                    # tail tiles: halve the copy latency by splitting each
                    # PSUM->SBUF copy across both copy-capable engines
                    nc.vector.tensor_copy(y_sb[:, :half], py[:, :half])
                    nc.scalar.copy(y_sb[:, half:], py[:, half:])
                else:
                    nc.vector.tensor_copy(y_sb[:], py[:])
                o_sb = opool.tile([128, 4, free], bf16, tag="o")
                for ob in range(4):
                    obs = slice(ob * 128, (ob + 1) * 128)
                    po = psum_o_pool.tile([128, free], f32, tag="po")
                    nc.tensor.matmul(po[:], m_sb[:, obs], y_sb[:],
                                     start=True, stop=True)
                    if fast and fast_mode in ("out", "yout"):
                        nc.vector.tensor_copy(o_sb[:, ob, :half],
                                              po[:, :half])
                        nc.scalar.copy(o_sb[:, ob, half:], po[:, half:])
                    elif ob % 2 == 0:
                        nc.vector.tensor_copy(o_sb[:, ob, :], po[:])
                    else:
                        nc.scalar.copy(o_sb[:, ob, :], po[:])
                    if mode == "split":
                        nc.sync.dma_start(
                            out_d[:, off + ob * free:off + (ob + 1) * free],
                            o_sb[:, ob, :])
                    elif mode == "pairs" and ob % 2 == 1:
                        # pairwise DMAs decouple the transfer start from the
                        # slowest of all 4 copies without doubling HWDGE load
                        nc.sync.dma_start(
                            out_d[:, off + (ob - 1) * free:
                                  off + (ob + 1) * free],
                            o_sb[:, ob - 1:ob + 1, :])
                if mode == "merged":
                    nc.sync.dma_start(
                        out_d[:, off:off + 4 * free], o_sb[:])

            tiles = _tiles(tiles1)
            n_total = len(tiles[0]) + len(tiles[1])

            # tail= "<mid>-<lastk>-<last>": DMA mode for mid tiles, for the
            # 2 next-to-last tiles, and for the final tile
            mid_mode, lastk_mode, last_mode = tail.split("-")

            # software-pipeline by one tile: PE runs tile i's conv while
            # ACT/DVE copy tile i-1's PSUM out, so the mix matmuls are
            # ready when PE gets to them
            pending = None
            emitted = 0
            for img in range(IMGS_PER_CORE):
                for h0, nr in tiles[img]:
                    py = emit_conv(img, h0, nr)
                    if pending is not None:
                        emitted += 1
                        mode = (mid_mode if emitted < n_total - 2
                                else lastk_mode)
                        emit_mix(*pending, mode=mode,
                                 fast=emitted >= n_total - 1)
                    pending = (py, img, h0, nr)
            emit_mix(*pending, mode=last_mode, fast=True)

    nc.compile()
    return nc


_NC_CACHE = None


def kernel(x, dictionary, lookup_indices, lookup_coefficients):
    global _NC_CACHE
    from concourse import bass_utils

    x = np.asarray(x, dtype=np.float32)
    dictionary = np.asarray(dictionary, dtype=np.float32)
    idx = np.asarray(lookup_indices).astype(np.int64)
    coef = np.asarray(lookup_coefficients, dtype=np.float32)

    # M^T[d, o] = sum_s coeff[o, s] * [idx[o, s] == d]
    mt = np.zeros((NDICT, COUT), np.float32)
    np.add.at(mt, (idx.reshape(-1),
                   np.repeat(np.arange(COUT), S)), coef.reshape(-1))

    # wt[c_in_block, (cblk, ti, tj, d)] = dictionary[d, cblk*128+c, ti, tj]
    wt = np.ascontiguousarray(
        dictionary.reshape(NDICT, 2, 128, 3, 3).transpose(2, 1, 3, 4, 0)
    ).reshape(128, 2 * 9 * NDICT)

    xp = np.pad(x, ((0, 0), (0, 0), (1, 1), (1, 1)))
    xp = np.ascontiguousarray(
        xp.reshape(N_CORES, IMGS_PER_CORE, CIN, HP, WP))

    bf = ml_dtypes.bfloat16
    xb = xp.astype(bf)
    wb = wt.astype(bf)
    mb = mt.astype(bf)

    if _NC_CACHE is None:
        _NC_CACHE = _build_program()
    nc = _NC_CACHE

    in_maps = [{"x": xb[i], "w": wb, "m": mb} for i in range(N_CORES)]
    try:
        res = bass_utils.run_bass_kernel_spmd(
            nc, in_maps, core_ids=list(range(N_CORES)), trace=TRACE)
    except ModuleNotFoundError:
        # no axon NTFF profile hook in this environment
        res = bass_utils.run_bass_kernel_spmd(
            nc, in_maps, core_ids=list(range(N_CORES)), trace=False)
    _LAST_RESULTS["res"] = res

    # untangle the tile-major device layout [o, 4*(img*3136+h0*56)+ob*f+px]
    tiles = _tiles("44")
    out = np.empty((N_CORES, IMGS_PER_CORE, COUT, H, W), np.float32)
    for c, r in enumerate(res.results):
        arr = np.asarray(r["out"])  # [128, 4*2*3136] bf16
        for img in range(IMGS_PER_CORE):
            for h0, nr in tiles[img]:
                off = 4 * (img * H * W + h0 * W)
                seg = arr[:, off:off + 4 * nr * W].astype(np.float32)
                seg = seg.reshape(128, 4, nr, W).transpose(1, 0, 2, 3)
                out[c, img, :, h0:h0 + nr, :] = seg.reshape(COUT, nr, W)
    return out.reshape(16, COUT, H, W)


# revision 39
# speedup vs baseline: 2.8761x; 1.0009x over previous
"""LookupConv2d Trainium2 kernel.

Math: out = conv2d(x, W), W[o] = sum_s coeff[o,s] * dictionary[idx[o,s]].
Factorization: W = M @ D where M[o,d] = sum_{s: idx[o,s]=d} coeff[o,s] is a
(512, 100) scatter of the coefficients.  Then
    out = M @ conv2d(x, dictionary)
i.e. a 100-channel conv (23 GFLOP) followed by a 1x1 512x100 mix (5 GFLOP)
instead of a 512-channel conv (118 GFLOP) -- 4.2x fewer FLOPs.

Precision: single-pass bf16.  All matmul operands are bf16 (streamed at
1 column/cycle on TensorE, same rate as f32r), accumulation is fp32 in
PSUM.  Simulated end-to-end rel err 3.6e-3, well under the 2e-2 gate.
I/O is bf16 as well (input 3.45 MB, output 6.4 MB per core), halving HBM
traffic vs fp32.

Sharding: data-parallel over batch N=16 -> 2 images per core on 8 cores.
dictionary (as [128, 1800] bf16 tap matrices) and M^T are replicated.
"""

import numpy as np
import ml_dtypes

N_CORES = 8
IMGS_PER_CORE = 2
CIN = 256
COUT = 512
NDICT = 100
H = W = 56
HP = WP = 58  # padded
ROWS_PER_TILE = 8
N_TILES = H // ROWS_PER_TILE  # 7
FREE = ROWS_PER_TILE * W  # 448
S = 3  # lookup sparsity

TRACE = False  # set by test.py to get a profile
_LAST_RESULTS = {}  # test.py reads exec_time_ns from here


def _tiles(tiles1="8"):
    """Row-tile grid per image.  img0 starts with two 4-row tiles so
    compute can begin after only 6 input rows + 3 taps land; img1's tail
    can end with small tiles to shorten the kernel tail."""
    t0 = [(0, 4), (4, 4)] + [(8 + 8 * t, 8) for t in range(6)]
    if tiles1 == "44":
        t1 = [(8 * t, 8) for t in range(6)] + [(48, 4), (52, 4)]
    elif tiles1 == "62":
        t1 = [(8 * t, 8) for t in range(6)] + [(48, 6), (54, 2)]
    else:
        t1 = [(8 * t, 8) for t in range(7)]
    return {0: t0, 1: t1}


def _build_program(head="C", tail="pairs-merged-merged", tiles1="44",
                   fast_mode="none"):
    import concourse.bacc as bacc
    import concourse.mybir as mybir
    import concourse.tile as tile

    f32 = mybir.dt.float32
    bf16 = mybir.dt.bfloat16

    nc = bacc.Bacc("TRN2", target_bir_lowering=False, debug=False)

    x_d = nc.dram_tensor("x", (IMGS_PER_CORE, CIN, HP, WP), bf16,
                         kind="ExternalInput")
    w_d = nc.dram_tensor("w", (128, 2 * 9 * NDICT), bf16, kind="ExternalInput")
    m_d = nc.dram_tensor("m", (NDICT, COUT), bf16, kind="ExternalInput")
    # tile-major output layout out[o, 4*(img*3136 + h0*56) + ob*free + px]:
    # every out-DMA writes a per-partition-contiguous run (>=512 B even for
    # tiny tail tiles), so all transfers go at full HBM line rate.  The
    # host untangles this for free.
    out_d = nc.dram_tensor("out", (128, 4 * H * W * IMGS_PER_CORE), bf16,
                           kind="ExternalOutput")

    with tile.TileContext(nc) as tc:
        with (
            tc.tile_pool(name="consts", bufs=1) as consts,
            tc.tile_pool(name="xpool", bufs=1) as xpool,
            tc.tile_pool(name="ypool", bufs=3) as ypool,
            tc.tile_pool(name="opool", bufs=3) as opool,
            tc.tile_pool(name="psum_y", bufs=2, space="PSUM") as psum_y_pool,
            tc.tile_pool(name="psum_o", bufs=4, space="PSUM") as psum_o_pool,
        ):
            w_sb = consts.tile([128, 2 * 9 * NDICT], bf16)
            m_sb = consts.tile([NDICT, COUT], bf16)
            # [128 cin-in-block, img, cblk, hp, wp]
            x_sb = xpool.tile([128, IMGS_PER_CORE, 2, HP, WP], bf16,
                              tag="x_sb")
            x_v = x_d.rearrange("i (b c) h w -> c i b h w", c=128)

            # prologue DMAs, ordered by when compute needs them: the first
            # 3 conv matmuls only need the first 3 cb=0 taps and rows 0..9
            # of image 0.  Few large transfers keep the HWDGE (fixed
            # ~625ns per dma_start) far from saturation.
            def d_x(img, cb, r0, r1):
                if cb is None:
                    nc.sync.dma_start(x_sb[:, img, :, r0:r1, :],
                                      x_v[:, img, :, r0:r1, :])
                else:
                    nc.sync.dma_start(x_sb[:, img, cb, r0:r1, :],
                                      x_v[:, img, cb, r0:r1, :])

            def d_w(c0, c1):
                nc.sync.dma_start(w_sb[:, c0 * NDICT:c1 * NDICT],
                                  w_d[:, c0 * NDICT:c1 * NDICT])

            def d_m():
                nc.sync.dma_start(m_sb[:], m_d[:])

            heads = {
                # x img0 rows first-chunk / weight pieces / m, in need order
                "A": lambda: (d_x(0, 0, 0, 6), d_w(0, 3), d_x(0, 1, 0, 6),
                              d_x(0, None, 6, 10), d_w(3, 9), d_w(9, 18),
                              d_m()),
                "B": lambda: (d_x(0, 0, 0, 6), d_w(0, 3), d_x(0, 1, 0, 6),
                              d_w(3, 9), d_w(9, 18), d_x(0, None, 6, 10),
                              d_m()),
                "C": lambda: (d_x(0, 0, 0, 6), d_w(0, 3), d_w(3, 9),
                              d_x(0, 1, 0, 6), d_w(9, 18),
                              d_x(0, None, 6, 10), d_m()),
                "D": lambda: (d_x(0, 0, 0, 10), d_w(0, 3), d_x(0, 1, 0, 10),
                              d_w(3, 9), d_w(9, 18), d_m()),
                "E": lambda: (d_x(0, 0, 0, 6), d_w(0, 3), d_x(0, 1, 0, 6),
                              d_w(3, 9), d_x(0, None, 6, 10), d_w(9, 18),
                              d_m()),
                "F": lambda: (d_x(0, 0, 0, 6), d_w(0, 6), d_x(0, 1, 0, 6),
                              d_w(6, 12), d_x(0, None, 6, 10), d_w(12, 18),
                              d_m()),
            }
            heads[head]()
            late_chunks = [(10, 24), (34, 24)]
            for r0, nr in late_chunks:
                d_x(0, None, r0, r0 + nr)
            for r0, nr in [(0, 10)] + late_chunks:
                d_x(1, None, r0, r0 + nr)



            def emit_conv(img, h0, nr):
                free = nr * W
                py = psum_y_pool.tile([NDICT, free], f32, tag="py")
                k = 0
                for cb in range(2):
                    for ti in range(3):
                        for tj in range(3):
                            tap = slice((cb * 9 + ti * 3 + tj) * NDICT,
                                        (cb * 9 + ti * 3 + tj + 1) * NDICT)
                            rh = (slice(None), img, cb,
                                  slice(h0 + ti, h0 + ti + nr),
                                  slice(tj, tj + W))
                            nc.tensor.matmul(
                                py[:], w_sb[:, tap], x_sb[rh],
                                start=(k == 0), stop=(k == 17))
                            k += 1
                return py

            def emit_mix(py, img, h0, nr, mode="pairs", fast=False):
                free = nr * W
                half = free // 2
                off = 4 * (img * H * W + h0 * W)
                y_sb = ypool.tile([NDICT, free], bf16, tag="y")
                if fast and fast_mode == "yout":
                    # tail tiles: halve the copy latency by splitting each
                    # PSUM->SBUF copy across both copy-capable engines
                    nc.vector.tensor_copy(y_sb[:, :half], py[:, :half])
                    nc.scalar.copy(y_sb[:, half:], py[:, half:])
                else:
                    nc.vector.tensor_copy(y_sb[:], py[:])
                o_sb = opool.tile([128, 4, free], bf16, tag="o")
                for ob in range(4):
                    obs = slice(ob * 128, (ob + 1) * 128)
                    po = psum_o_pool.tile([128, free], f32, tag="po")
                    nc.tensor.matmul(po[:], m_sb[:, obs], y_sb[:],
                                     start=True, stop=True)
                    if fast and fast_mode in ("out", "yout"):
                        nc.vector.tensor_copy(o_sb[:, ob, :half],
                                              po[:, :half])
                        nc.scalar.copy(o_sb[:, ob, half:], po[:, half:])
                    elif ob % 2 == 0:
                        nc.vector.tensor_copy(o_sb[:, ob, :], po[:])
                    else:
                        nc.scalar.copy(o_sb[:, ob, :], po[:])
                    if mode == "split":
                        nc.sync.dma_start(
                            out_d[:, off + ob * free:off + (ob + 1) * free],
                            o_sb[:, ob, :])
                    elif mode == "pairs" and ob % 2 == 1:
                        # pairwise DMAs decouple the transfer start from the
                        # slowest of all 4 copies without doubling HWDGE load
                        nc.sync.dma_start(
                            out_d[:, off + (ob - 1) * free:
                                  off + (ob + 1) * free],
                            o_sb[:, ob - 1:ob + 1, :])
                if mode == "merged":
                    nc.sync.dma_start(
                        out_d[:, off:off + 4 * free], o_sb[:])

            tiles = _tiles(tiles1)
            n_total = len(tiles[0]) + len(tiles[1])

            # tail= "<mid>-<lastk>-<last>": DMA mode for mid tiles, for the
            # 2 next-to-last tiles, and for the final tile
            mid_mode, lastk_mode, last_mode = tail.split("-")

            # software-pipeline by one tile: PE runs tile i's conv while
            # ACT/DVE copy tile i-1's PSUM out, so the mix matmuls are
            # ready when PE gets to them
            pending = None
            emitted = 0
            for img in range(IMGS_PER_CORE):
                for h0, nr in tiles[img]:
                    py = emit_conv(img, h0, nr)
                    if pending is not None:
                        emitted += 1
                        mode = (mid_mode if emitted < n_total - 2
                                else lastk_mode)
                        emit_mix(*pending, mode=mode,
                                 fast=emitted >= n_total - 1)
                    pending = (py, img, h0, nr)
            emit_mix(*pending, mode=last_mode, fast=True)

    nc.compile()
    return nc


_NC_CACHE = None


def kernel(x, dictionary, lookup_indices, lookup_coefficients):
    global _NC_CACHE
    from concourse import bass_utils

    x = np.asarray(x, dtype=np.float32)
    dictionary = np.asarray(dictionary, dtype=np.float32)
    idx = np.asarray(lookup_indices).astype(np.int64)
    coef = np.asarray(lookup_coefficients, dtype=np.float32)

    # M^T[d, o] = sum_s coeff[o, s] * [idx[o, s] == d]
    mt = np.zeros((NDICT, COUT), np.float32)
    np.add.at(mt, (idx.reshape(-1),
                   np.repeat(np.arange(COUT), S)), coef.reshape(-1))

    # wt[c_in_block, (cblk, ti, tj, d)] = dictionary[d, cblk*128+c, ti, tj]
    wt = np.ascontiguousarray(
        dictionary.reshape(NDICT, 2, 128, 3, 3).transpose(2, 1, 3, 4, 0)
    ).reshape(128, 2 * 9 * NDICT)

    xp = np.pad(x, ((0, 0), (0, 0), (1, 1), (1, 1)))
    xp = np.ascontiguousarray(
        xp.reshape(N_CORES, IMGS_PER_CORE, CIN, HP, WP))

    bf = ml_dtypes.bfloat16
    xb = xp.astype(bf)
    wb = wt.astype(bf)
    mb = mt.astype(bf)

    if _NC_CACHE is None:
        _NC_CACHE = _build_program()
    nc = _NC_CACHE

    in_maps = [{"x": xb[i], "w": wb, "m": mb} for i in range(N_CORES)]
    try:
        res = bass_utils.run_bass_kernel_spmd(
            nc, in_maps, core_ids=list(range(N_CORES)), trace=TRACE)
    except ModuleNotFoundError:
        # no axon NTFF profile hook in this environment
        res = bass_utils.run_bass_kernel_spmd(
            nc, in_maps, core_ids=list(range(N_CORES)), trace=False)
    _LAST_RESULTS["res"] = res

    # untangle the tile-major device layout [o, 4*(img*3136+h0*56)+ob*f+px]
    tiles = _tiles("44")
    out = np.empty((N_CORES, IMGS_PER_CORE, COUT, H, W), np.float32)
    for c, r in enumerate(res.results):
        arr = np.asarray(r["out"])  # [128, 4*2*3136] bf16
        for img in range(IMGS_PER_CORE):
            for h0, nr in tiles[img]:
                off = 4 * (img * H * W + h0 * W)
                seg = arr[:, off:off + 4 * nr * W].astype(np.float32)
                seg = seg.reshape(128, 4, nr, W).transpose(1, 0, 2, 3)
                out[c, img, :, h0:h0 + nr, :] = seg.reshape(COUT, nr, W)
    return out.reshape(16, COUT, H, W)
